# revision 9
# baseline (speedup 1.0000x reference)
"""Multi-head attention kernel for Trainium2, 8 NeuronCores, data-parallel over batch.

Problem (matches the reference nn.Module):
  B=8, S=1024, D_IN=D_OUT=1024, H=16, D_K=64, fp32 in/out.
  q/k/v = Linear(x) per input; scores = q k^T / sqrt(64); attn = softmax;
  out = (attn v) heads-concatenated -> [B, S*D_OUT].

Strategy:
  - One batch element per core (8 cores). No collectives.
  - All matmul operands bf16 (PSUM f32). Host pre-transposes inputs.
  - On-chip layouts:
      kT/qT [P, KT, S]  (o = p + 128*t; head h lives at t=h//2,
                         partitions (h%2)*64..+64)
      vP    [P, KT, H*(DK+1)]  per-head 65-col groups, col 64 = 1.0 so
                         the PV matmul also emits the softmax denominator.
  - Scores are emitted as HEAD PAIRS: even head h uses array rows 0-63,
    odd head h+1 rows 64-127 (tile_position auto-derived from base
    partitions) -> the two 64-contraction matmuls run CONCURRENTLY in
    the PE array (row tiling), ~2x scores throughput.
  - attn^T = exp(scores^T/8) on ACT; ACT is the near-bottleneck
    (~147us of exp), so the schedule starts exp as early as possible:
    warmup MMs trip the HAM clock gate + preload the exp table while
    the first K/Q o-tiles stream in; scores for head pair 0 start after
    only 3 projection blocks; all remaining projection blocks are
    interleaved between scores steps to keep the PE busy while ACT
    drains the exp backlog.
  - pv[65, q] = V'_h.T @ attn^T accumulated over 8 k-tiles; raw blocks
    (incl. denominator row 64) are DMA'd out; the final transpose to
    [q, h*64+d] and the divide happen on the HOST.
"""

import numpy as np

B = 8
S = 1024
D = 1024          # D_IN == D_OUT
H = 16
DK = 64           # D_K
KT = 8            # 128-row tiles along a 1024 dim
QC = 2            # q-chunks of 512
P = 128
NCH = 512         # matmul moving free dim
PV_LAG = 3        # pair-iterations between scores emission and PV
N_WARM = 10       # HAM warmup matmuls (~4.3us @ cold clock)

_cache = {}


def _build():
    import concourse.tile as tile
    import concourse.mybir as mybir
    from concourse import bacc

    F32 = mybir.dt.float32
    BF16 = mybir.dt.bfloat16
    Exp = mybir.ActivationFunctionType.Exp
    MMDT = BF16

    nc = bacc.Bacc(None, target_bir_lowering=False, debug=True)

    xqT = nc.declare_dram_parameter("xqT", [D, S], BF16, isOutput=False)
    xkT = nc.declare_dram_parameter("xkT", [D, S], BF16, isOutput=False)
    xvT = nc.declare_dram_parameter("xvT", [D, S], BF16, isOutput=False)
    wqT = nc.declare_dram_parameter("wqT", [D, D], BF16, isOutput=False)
    wkT = nc.declare_dram_parameter("wkT", [D, D], BF16, isOutput=False)
    wvT = nc.declare_dram_parameter("wvT", [D, D], BF16, isOutput=False)
    bq = nc.declare_dram_parameter("bq", [D], F32, isOutput=False)
    bk = nc.declare_dram_parameter("bk", [D], F32, isOutput=False)
    bv = nc.declare_dram_parameter("bv", [D], F32, isOutput=False)
    # raw per-head PV output incl. denominator row; host divides+transposes
    out = nc.declare_dram_parameter("out", [H, DK + 1, S], F32, isOutput=True)

    with tile.TileContext(nc) as tc:
        with tc.tile_pool(name="persist", bufs=1) as persist, \
             tc.tile_pool(name="attn_p", bufs=2 * (PV_LAG + 1)) as attn_p, \
             tc.tile_pool(name="ot_p", bufs=3) as ot_p, \
             tc.tile_pool(name="xk_p", bufs=2) as xk_p, \
             tc.tile_pool(name="xq_p", bufs=2) as xq_p, \
             tc.tile_pool(name="xv_p", bufs=2) as xv_p, \
             tc.tile_pool(name="w_p", bufs=4) as w_p, \
             tc.tile_pool(name="b_p", bufs=1) as b_p, \
             tc.tile_pool(name="wu_p", bufs=1) as wu_p, \
             tc.tile_pool(name="pp", bufs=2, space="PSUM") as pp, \
             tc.tile_pool(name="sc_p", bufs=2, space="PSUM") as sc_p, \
             tc.tile_pool(name="pv_p", bufs=2, space="PSUM") as pv_p:

            qT = persist.tile([P, KT, S], MMDT, tag="qT")
            kT = persist.tile([P, KT, S], MMDT, tag="kT")
            vP = persist.tile([P, KT, H * (DK + 1)], MMDT, tag="vP")

            # ---- input tile caches (DMA issued at first touch) ----
            xsrc = {"q": xqT, "k": xkT, "v": xvT}
            wsrc = {"q": wqT, "k": wkT, "v": wvT}
            xpool = {"q": xq_p, "k": xk_p, "v": xv_p}
            xtiles, wtiles = {}, {}

            def get_x(kind, sc):
                key = (kind, sc)
                if key not in xtiles:
                    t_ = xpool[kind].tile([P, KT, NCH], MMDT, tag=f"x{kind}",
                                          name=f"x_{kind}{sc}")
                    src = xsrc[kind][:].rearrange("(t p) s -> p t s", p=P)
                    for t in range(KT):
                        nc.sync.dma_start(
                            out=t_[:, t, :],
                            in_=src[:, t, sc * NCH:(sc + 1) * NCH])
                    xtiles[key] = t_
                return xtiles[key]

            def get_w(kind, oh):
                key = (kind, oh)
                if key not in wtiles:
                    t_ = w_p.tile([P, KT, NCH], MMDT, tag="w",
                                  name=f"w_{kind}{oh}")
                    src = wsrc[kind][:].rearrange("(t p) s -> p t s", p=P)
                    for t in range(KT):
                        nc.sync.dma_start(
                            out=t_[:, t, :],
                            in_=src[:, t, oh * NCH:(oh + 1) * NCH])
                    wtiles[key] = t_
                return wtiles[key]

            # prefetch the pre-roll inputs before anything else queues
            get_x("k", 0), get_w("k", 0), get_x("k", 1)
            get_x("q", 0), get_w("q", 0)

            # ---- biases ----
            bqs = b_p.tile([P, KT], F32, tag="bqs")
            bks = b_p.tile([P, KT], F32, tag="bks")
            nc.sync.dma_start(out=bqs[:], in_=bq[:].rearrange("(t p) -> p t", p=P))
            nc.sync.dma_start(out=bks[:], in_=bk[:].rearrange("(t p) -> p t", p=P))
            bvb = b_p.tile([P, D], F32, tag="bvb")
            nc.gpsimd.dma_start(out=bvb[:], in_=bv[:].partition_broadcast(P))

            # ones columns of V' (V-block writes skip col 64 of each group)
            ones16 = b_p.tile([P, H], F32, tag="ones16")
            nc.vector.memset(ones16[:], 1.0)
            for st in range(KT):
                nc.vector.tensor_copy(
                    out=vP[:, st, :]
                    .rearrange("p (h d) -> p h d", h=H)[:, :, DK:DK + 1],
                    in_=ones16[:].unsqueeze(2),
                )

            # ---- HAM warmup + exp table preload ----
            wu = wu_p.tile([P, NCH], MMDT, tag="wu")
            nc.vector.memset(wu[:], 0.0)
            wuf = wu_p.tile([P, 8], F32, tag="wuf")
            nc.vector.memset(wuf[:], 0.0)
            wue = wu_p.tile([P, 8], F32, tag="wue")
            nc.scalar.activation(out=wue[:], in_=wuf[:], func=Exp, scale=1.0)
            for i in range(N_WARM):
                ps_ = pp.tile([P, NCH], F32, tag="proj", name=f"warm{i}")
                nc.tensor.matmul(ps_[:], wu[:, 0:P], wu[:],
                                 start=True, stop=True)

            # ---- projection blocks ----
            def kq_block(kind, ot, sc):
                dst, bias = (qT, bqs) if kind == "q" else (kT, bks)
                w_sb = get_w(kind, ot // 4)
                x_sb = get_x(kind, sc)
                o4 = ot % 4
                ps_ = pp.tile([P, NCH], F32, tag="proj",
                              name=f"ps_{kind}{ot}{sc}")
                for it in range(KT):
                    nc.tensor.matmul(
                        ps_[:],
                        w_sb[:, it, o4 * P:(o4 + 1) * P],
                        x_sb[:, it, :],
                        start=(it == 0), stop=(it == KT - 1))
                nc.vector.tensor_scalar_add(
                    out=dst[:, ot, sc * NCH:(sc + 1) * NCH],
                    in0=ps_[:], scalar1=bias[:, ot:ot + 1])

            def v_block(oh, sc, s4):
                w_sb = get_w("v", oh)
                x_sb = get_x("v", sc)
                st = sc * 4 + s4
                ps_ = pp.tile([P, NCH], F32, tag="proj", name=f"ps_v{oh}{st}")
                for it in range(KT):
                    nc.tensor.matmul(
                        ps_[:],
                        x_sb[:, it, s4 * P:(s4 + 1) * P],
                        w_sb[:, it, :],
                        start=(it == 0), stop=(it == KT - 1))
                nc.vector.tensor_tensor(
                    out=vP[:, st, :]
                    .rearrange("p (h d) -> p h d", h=H)[:, oh * 8:(oh + 1) * 8, 0:DK],
                    in0=ps_[:].rearrange("p (h d) -> p h d", h=8),
                    in1=bvb[:, oh * NCH:(oh + 1) * NCH]
                    .rearrange("p (h d) -> p h d", h=8),
                    op=mybir.AluOpType.add)

            def run_block(bid):
                if bid[0] == "v":
                    v_block(bid[1], bid[2], bid[3])
                else:
                    kq_block(bid[0], bid[1], bid[2])
                done_ids.add(bid)

            # pre-roll blocks emitted before the first scores step
            pre = [("k", 0, 0), ("k", 0, 1), ("q", 0, 0)]
            queue = [("q", 0, 1)]
            for ot in (1,):
                queue += [("k", ot, 0), ("k", ot, 1), ("q", ot, 0), ("q", ot, 1)]
            queue += [("v", 0, sc, s4) for sc in (0, 1) for s4 in range(4)]
            for ot in (2, 3, 4):
                queue += [("k", ot, 0), ("k", ot, 1), ("q", ot, 0), ("q", ot, 1)]
            queue += [("v", 1, sc, s4) for sc in (0, 1) for s4 in range(4)]
            for ot in (5, 6, 7):
                queue += [("k", ot, 0), ("k", ot, 1), ("q", ot, 0), ("q", ot, 1)]

            done_ids = set()
            qpos = [0]

            def drain_n(n):
                took = 0
                while qpos[0] < len(queue) and took < n:
                    run_block(queue[qpos[0]])
                    qpos[0] += 1
                    took += 1

            def drain_through(ids):
                while any(i not in done_ids for i in ids):
                    assert qpos[0] < len(queue), f"missing {ids}"
                    run_block(queue[qpos[0]])
                    qpos[0] += 1

            for b in pre:
                run_block(b)

            # ---- attention emission ----
            iters = [(hp, qc) for hp in range(H // 2) for qc in range(QC)]
            attns = {}

            def sc_step(ap, hp, qc, kb):
                if kb == 0:
                    attns[ap] = (
                        attn_p.tile([P, KT, NCH], MMDT, tag="attnT",
                                    name=f"aA{ap}"),
                        attn_p.tile([P, KT, NCH], MMDT, tag="attnT",
                                    name=f"aB{ap}"),
                    )
                tA, tB = attns[ap]
                At = sc_p.tile([P, 2, NCH], F32, tag="sc", name=f"scA{ap}_{kb}")
                Bt = sc_p.tile([P, 2, NCH], F32, tag="sc", name=f"scB{ap}_{kb}")
                q_lo = qT[0:DK, hp, qc * NCH:(qc + 1) * NCH]
                q_hi = qT[DK:P, hp, qc * NCH:(qc + 1) * NCH]
                for j in (0, 1):
                    kt = 2 * kb + j
                    # even head: array rows 0-63; odd head: rows 64-127.
                    # Emitted adjacently -> the PE runs them concurrently.
                    nc.tensor.matmul(
                        At[:, j, :],
                        kT[0:DK, hp, kt * P:(kt + 1) * P], q_lo,
                        start=True, stop=True)
                    nc.tensor.matmul(
                        Bt[:, j, :],
                        kT[DK:P, hp, kt * P:(kt + 1) * P], q_hi,
                        start=True, stop=True)
                nc.scalar.activation(out=tA[:, 2 * kb:2 * kb + 2, :],
                                     in_=At[:], func=Exp, scale=0.125)
                nc.scalar.activation(out=tB[:, 2 * kb:2 * kb + 2, :],
                                     in_=Bt[:], func=Exp, scale=0.125)

            def emit_out(qc, h, attnT):
                pv = pv_p.tile([DK + 1, NCH], F32, tag="pv",
                               name=f"pv{qc}_{h}")
                for kt in range(KT):
                    nc.tensor.matmul(
                        pv[:],
                        vP[:, kt, h * (DK + 1):(h + 1) * (DK + 1)],
                        attnT[:, kt, :],
                        start=(kt == 0), stop=(kt == KT - 1))
                ot_sb = ot_p.tile([DK + 1, NCH], F32, tag="ot",
                                  name=f"ot{qc}_{h}")
                nc.vector.tensor_copy(out=ot_sb[:], in_=pv[:])
                nc.sync.dma_start(
                    out=out[h, :, qc * NCH:(qc + 1) * NCH], in_=ot_sb[:])

            def pv_pair(ap):
                hp, qc = iters[ap]
                oh = hp // 4
                drain_through([("v", oh, sc, s4)
                               for sc in (0, 1) for s4 in range(4)])
                tA, tB = attns.pop(ap)
                emit_out(qc, 2 * hp, tA)
                emit_out(qc, 2 * hp + 1, tB)

            for ap, (hp, qc) in enumerate(iters):
                drain_through([("k", hp, 0), ("k", hp, 1), ("q", hp, qc)])
                for kb in range(KT // 2):
                    sc_step(ap, hp, qc, kb)
                    drain_n(1)
                if ap >= PV_LAG:
                    pv_pair(ap - PV_LAG)
            drain_n(len(queue))
            for ap in range(len(iters) - PV_LAG, len(iters)):
                pv_pair(ap)
            assert qpos[0] == len(queue) and not attns

    nc.finalize()
    return nc


def _get_program():
    key = "prog"
    if key not in _cache:
        _cache[key] = _build()
    return _cache[key]


def _prep_in_maps(inputs):
    import ml_dtypes

    BF = ml_dtypes.bfloat16
    query = np.asarray(inputs["query"], dtype=np.float32)
    key_ = np.asarray(inputs["key_"], dtype=np.float32)
    value = np.asarray(inputs["value"], dtype=np.float32)
    wqT = np.ascontiguousarray(np.asarray(inputs["Wq"], dtype=np.float32).T.astype(BF))
    wkT = np.ascontiguousarray(np.asarray(inputs["Wk"], dtype=np.float32).T.astype(BF))
    wvT = np.ascontiguousarray(np.asarray(inputs["Wv"], dtype=np.float32).T.astype(BF))
    bq = np.ascontiguousarray(np.asarray(inputs["bq"], dtype=np.float32))
    bk = np.ascontiguousarray(np.asarray(inputs["bk"], dtype=np.float32))
    bv = np.ascontiguousarray(np.asarray(inputs["bv"], dtype=np.float32))
    return [
        {
            "xqT": np.ascontiguousarray(query[b].T.astype(BF)),
            "xkT": np.ascontiguousarray(key_[b].T.astype(BF)),
            "xvT": np.ascontiguousarray(value[b].T.astype(BF)),
            "wqT": wqT, "wkT": wkT, "wvT": wvT,
            "bq": bq, "bk": bk, "bv": bv,
        }
        for b in range(B)
    ]


def kernel(query, key_, value, Wq, bq, Wk, bk, Wv, bv):
    from concourse.bass_utils import run_bass_kernel_spmd

    nc = _get_program()
    in_maps = _prep_in_maps(dict(
        query=query, key_=key_, value=value,
        Wq=Wq, bq=bq, Wk=Wk, bk=bk, Wv=Wv, bv=bv,
    ))
    res = run_bass_kernel_spmd(nc, in_maps, list(range(B)))
    out = np.empty((B, S * D), dtype=np.float32)
    for b in range(B):
        o = np.asarray(res.results[b]["out"])          # [H, DK+1, S]
        x = o[:, :DK, :] / o[:, DK:DK + 1, :]          # [H, DK, S]
        out[b] = x.transpose(2, 0, 1).reshape(-1)      # [S, H*DK] flattened
    return out
